# revision 17
# baseline (speedup 1.0000x reference)
"""Multi-head attention forward (B=16, S=1024, d=1024, H=16, Dh=64) on 8
Trainium2 NeuronCores, data-parallel over batch (2 batches per core).

v2: fp16 matmuls (fp32 accumulate), explicit PE-stream scheduling.
  - scores per head-pair j: two K=64 matmuls on PE row-groups (0,0)/(64,0)
    co-issue (hardware tile concurrency) -> 2x on the score phase.
  - score PSUM pool is 3 banks so score matmuls never serialize behind exp.
  - all projections (Q/K/V/O) are 8-matmul single-bank accumulation chains
    interleaved between score/PV matmuls so LDWEIGHTS stays hidden and the
    scalar engine (exp) is continuously fed.
  - PV accumulators (4 banks) drain to SBUF via fast DVE copies at the head
    boundary; normalize (1/denominator) happens off the critical PSUM path.

Device layout per core:
  XT [d, 2048] = hidden[2c:2c+2].reshape(2048,d).T  (fp16)
  WqT/WkT/WvT/WoT = W.T [in, out] fp16; bq, bk [1024] f32;
  bo2 = bo + Wo @ bv (bv folded: softmax rows sum to 1).
  QT/KT per pair j: [128, 1024];  V head-split with ones column;
  scoresT[s,t] = K @ Q.T; PT = exp(scoresT/8) fp16 (scores bounded);
  ctxT_aug[dv+1, t] accumulated in PSUM, row dv = softmax denominator.
  outT[o, t] = WoT.T @ ctxT (+bo2) -> host transposes back.
"""

import numpy as np
import ml_dtypes

import concourse.bass as bass
import concourse.mybir as mybir
import concourse.tile as tile
from concourse import bacc
from concourse.bass_utils import run_bass_kernel_spmd

P = 128
D = 1024
T = 2048  # tokens per core
TB = 1024  # tokens per batch (= S)
H = 16
DH = 64
KD = D // P  # 8 partition-tiles of the d/dv/s dims
NB = T // TB  # batches per core
NCORES = 8

F16 = mybir.dt.float16
F32 = mybir.dt.float32
EXPF = mybir.ActivationFunctionType.Exp
MULT = mybir.AluOpType.mult

# test.py hooks
TRACE = False
TRACE_KWARGS = {}
LAST_RESULTS = None
DEBUG_DUMPS = False

_NC_CACHE = None


def build_nc():
    nc = bacc.Bacc("TRN2", target_bir_lowering=False, debug=False, num_devices=NCORES)

    xt_d = nc.dram_tensor("xt", [D, T], F16, kind="ExternalInput")
    wqt_d = nc.dram_tensor("wqt", [D, D], F16, kind="ExternalInput")
    wkt_d = nc.dram_tensor("wkt", [D, D], F16, kind="ExternalInput")
    wvt_d = nc.dram_tensor("wvt", [D, D], F16, kind="ExternalInput")
    wot_d = nc.dram_tensor("wot", [D, D], F16, kind="ExternalInput")
    bq_d = nc.dram_tensor("bq", [D], F32, kind="ExternalInput")
    bk_d = nc.dram_tensor("bk", [D], F32, kind="ExternalInput")
    bo2_d = nc.dram_tensor("bo2", [D], F32, kind="ExternalInput")
    outt_d = nc.dram_tensor("outt", [D, T], F32, kind="ExternalOutput")
    if DEBUG_DUMPS:
        dbg = {
            "d_qt": nc.dram_tensor("d_qt", [P, TB], F16, kind="ExternalOutput"),
            "d_kt": nc.dram_tensor("d_kt", [P, TB], F16, kind="ExternalOutput"),
            "d_pt": nc.dram_tensor("d_pt", [P, TB], F16, kind="ExternalOutput"),
            "d_v0": nc.dram_tensor("d_v0", [P, H * (DH + 1)], F16, kind="ExternalOutput"),
            "d_u": nc.dram_tensor("d_u", [DH + 1, 512], F32, kind="ExternalOutput"),
            "d_ctx": nc.dram_tensor("d_ctx", [P, TB], F16, kind="ExternalOutput"),
        }

    with tile.TileContext(nc) as tc:
        from contextlib import ExitStack

        with ExitStack() as ctx:
            wpool = ctx.enter_context(tc.tile_pool(name="w", bufs=1))
            xpool = ctx.enter_context(tc.tile_pool(name="x", bufs=1))
            qkpool = ctx.enter_context(tc.tile_pool(name="qk", bufs=2))
            vpool = ctx.enter_context(tc.tile_pool(name="v", bufs=2))
            ptpool = ctx.enter_context(tc.tile_pool(name="pt", bufs=2))
            cpool = ctx.enter_context(tc.tile_pool(name="ctx", bufs=1))
            spool = ctx.enter_context(tc.tile_pool(name="small", bufs=1))
            npool = ctx.enter_context(tc.tile_pool(name="norm", bufs=2))
            upool = ctx.enter_context(tc.tile_pool(name="unorm", bufs=1))
            opool = ctx.enter_context(tc.tile_pool(name="out", bufs=1))
            psc = ctx.enter_context(tc.tile_pool(name="psc", bufs=3, space="PSUM"))
            ppv = ctx.enter_context(tc.tile_pool(name="ppv", bufs=1, space="PSUM"))
            pbr = ctx.enter_context(tc.tile_pool(name="pbr", bufs=1, space="PSUM"))

            # ---- global loads (biases first: needed by earliest drains) ----
            bq_sb = spool.tile([P, KD], F32, tag="bq", name="bq_sb")
            bk_sb = spool.tile([P, KD], F32, tag="bk", name="bk_sb")
            bo_sb = spool.tile([P, KD], F32, tag="bo", name="bo_sb")
            for sb, dd in ((bq_sb, bq_d), (bk_sb, bk_d), (bo_sb, bo2_d)):
                nc.sync.dma_start(sb[:], dd.rearrange("(o p) -> p o", p=P))

            xt = [xpool.tile([P, T], F16, tag=f"xt{k}", name=f"xt{k}") for k in range(KD)]
            wq, wk, wv, wo = (
                [wpool.tile([P, D], F16, tag=f"w{nm}{k}", name=f"w{nm}{k}") for k in range(KD)]
                for nm in "qkvo"
            )
            for k in range(KD):
                nc.sync.dma_start(wv[k][:], wvt_d[k * P : (k + 1) * P, :])
                nc.sync.dma_start(xt[k][:], xt_d[k * P : (k + 1) * P, :])
            for wt, wd in ((wq, wqt_d), (wk, wkt_d), (wo, wot_d)):
                for k in range(KD):
                    nc.sync.dma_start(wt[k][:], wd[k * P : (k + 1) * P, :])

            # ---- 8-matmul single-bank accumulation chains ----
            class Chain:
                def __init__(self, mk_mm, drain):
                    self.mk_mm = mk_mm
                    self.drain = drain
                    self.bank = "br"

                def run(self):
                    pool = pbr if self.bank == "br" else ppv
                    ps = pool.tile([P, 512], F32, tag=self.bank, name="chps")
                    for k in range(KD):
                        self.mk_mm(k, ps)
                    self.drain(ps)

            def qk_chains(b2, j2):
                qtj = qkpool.tile([P, TB], F16, tag="qtj", name="qtj")
                ktj = qkpool.tile([P, TB], F16, tag="ktj", name="ktj")
                chains = []
                for wt, bias, dest in ((wq, bq_sb, qtj), (wk, bk_sb, ktj)):
                    for c in range(2):
                        def mk(k, ps, wt=wt, c=c):
                            nc.tensor.matmul(
                                ps[:],
                                wt[k][:, j2 * P : (j2 + 1) * P],
                                xt[k][:, b2 * TB + c * 512 : b2 * TB + (c + 1) * 512],
                                start=(k == 0),
                                stop=(k == KD - 1),
                            )

                        def dr(ps, dest=dest, c=c, bias=bias):
                            nc.vector.tensor_scalar_add(
                                dest[:, c * 512 : (c + 1) * 512], ps[:], bias[:, j2 : j2 + 1]
                            )

                        chains.append(Chain(mk, dr))
                return (qtj, ktj), chains

            def v_chains(b2):
                vt = [
                    vpool.tile([P, H, DH + 1], F16, tag=f"v{mt}", name=f"v{mt}")
                    for mt in range(KD)
                ]
                for mt in range(KD):
                    nc.vector.memset(vt[mt][:, :, DH : DH + 1], 1.0)
                chains = []
                for mt in range(KD):
                    for c in range(2):
                        def mk(k, ps, mt=mt, c=c):
                            nc.tensor.matmul(
                                ps[:],
                                xt[k][:, (b2 * KD + mt) * P : (b2 * KD + mt + 1) * P],
                                wv[k][:, c * 512 : (c + 1) * 512],
                                start=(k == 0),
                                stop=(k == KD - 1),
                            )

                        def dr(ps, mt=mt, c=c):
                            nc.vector.tensor_copy(
                                vt[mt][:, c * 8 : (c + 1) * 8, 0:DH],
                                ps.rearrange("p (h d) -> p h d", d=DH),
                            )

                        chains.append(Chain(mk, dr))
                return vt, chains

            def out_chains(b2, ctile):
                chains = []
                for mo in range(KD):
                    for c in range(2):
                        def mk(k, ps, mo=mo, c=c):
                            nc.tensor.matmul(
                                ps[:],
                                wo[k][:, mo * P : (mo + 1) * P],
                                ctile[k][:, c * 512 : (c + 1) * 512],
                                start=(k == 0),
                                stop=(k == KD - 1),
                            )

                        def dr(ps, mo=mo, c=c):
                            osb = opool.tile([P, 512], F32, tag="osb", name="osb")
                            nc.vector.tensor_scalar_add(osb[:], ps[:], bo_sb[:, mo : mo + 1])
                            nc.sync.dma_start(
                                outt_d[
                                    mo * P : (mo + 1) * P,
                                    b2 * TB + c * 512 : b2 * TB + (c + 1) * 512,
                                ],
                                osb[:],
                            )

                        chains.append(Chain(mk, dr))
                return chains

            # ---- pending PV drain + normalize (emitted at next j start) ----
            def drain_norm(pend, dump=False):
                jp, pvt, ctile = pend
                us = {}
                for hh, c in ((0, 0), (1, 0), (0, 1), (1, 1)):
                    u = upool.tile([DH + 1, 512], F32, tag=f"u{hh}{c}", name="u")
                    nc.vector.tensor_copy(u[:], pvt[(hh, c)][0 : DH + 1, :])
                    us[(hh, c)] = u
                if dump:
                    nc.sync.dma_start(dbg["d_u"][:, :], us[(0, 0)][:])
                for hh, c in ((0, 0), (1, 0), (0, 1), (1, 1)):
                    u = us[(hh, c)]
                    # den row sits on partition 64; the custom-DVE reciprocal
                    # needs a partition-0 operand, so copy it down first
                    rs = npool.tile([1, 512], F32, tag="rs", name="rs", bufs=1)
                    nc.vector.tensor_copy(rs[:], u[DH : DH + 1, :])
                    rr = npool.tile([1, 512], F32, tag="rr", name="rr", bufs=1)
                    nc.vector.reciprocal_approx_fast(rr[:], rs[:])
                    rb = npool.tile([DH, 512], F32, tag="rb", name="rb", bufs=1)
                    nc.gpsimd.partition_broadcast(rb[:], rr[:])
                    if hh == 0:
                        nc.vector.tensor_tensor(
                            ctile[0:DH, c * 512 : (c + 1) * 512], u[0:DH, :], rb[:], MULT
                        )
                    else:
                        ch = npool.tile([DH, 512], F16, tag="ch", name="ch")
                        nc.vector.tensor_tensor(ch[:], u[0:DH, :], rb[:], MULT)
                        nc.sync.dma_start(
                            ctile[DH : 2 * DH, c * 512 : (c + 1) * 512], ch[:]
                        )

            qk_store = {}
            v_store = {}
            pending = None

            # ctxt tiles shared across batches: normalize(b+1) writes get a
            # WAR dependency on out-proj(b) reads of the same tile.
            ctxt = [
                cpool.tile([P, TB], F16, tag=f"ctxt{m}", name=f"ctxt{m}")
                for m in range(KD)
            ]

            # ---- prologue: v(b0) mt0 + qk(b0, j0), extra banks for pipelining ----
            vt0, vch0 = v_chains(0)
            v_store[0] = vt0
            qk_store[(0, 0)], qkch0 = qk_chains(0, 0)
            rest_v = vch0[2:]
            for i, chn in enumerate(vch0[0:2] + qkch0):
                chn.bank = ("br", "pv00", "pv10")[i % 3]
                chn.run()

            for b in range(NB):
                for j in range(KD):
                    qtj, ktj = qk_store.pop((b, j))
                    v = v_store[b]

                    # chain list for this j (order matters: earliest deadline first)
                    chl = []
                    if b == 0 and j == 0:
                        chl += rest_v
                    if j < KD - 1:
                        qk_store[(b, j + 1)], ch = qk_chains(b, j + 1)
                        chl += ch
                    elif b + 1 < NB:
                        qk_store[(b + 1, 0)], ch = qk_chains(b + 1, 0)
                        chl += ch
                    if j == 0 and b > 0:
                        chl += out_chains(b - 1, ctxt)
                    if b + 1 < NB and j == KD - 2:
                        vtn, vchn = v_chains(b + 1)
                        v_store[b + 1] = vtn
                        chl += vchn[:8]
                        v_store[(b + 1, "rest")] = vchn[8:]
                    if b + 1 < NB and j == KD - 1:
                        chl += v_store.pop((b + 1, "rest"))

                    # drain + normalize previous j BEFORE re-allocating the
                    # pv tags (reads of the old generation must be emitted
                    # before the slot is handed to the new one)
                    if pending is not None:
                        drain_norm(pending, dump=DEBUG_DUMPS and b == 0 and j == 1)
                        if DEBUG_DUMPS and b == 0 and j == 1:
                            nc.sync.dma_start(dbg["d_ctx"][:, :], ctxt[0][:])

                    # PV accumulators for this j
                    pvt = {
                        (hh, c): ppv.tile([P, 512], F32, tag=f"pv{hh}{c}", name="pv")
                        for hh in range(2)
                        for c in range(2)
                    }
                    pending = (j, pvt, ctxt[j])

                    pts = {}

                    def pv_mms(st2, pts=pts, pvt=pvt, v=v, j=j):
                        for hh, c in ((0, 0), (1, 0), (0, 1), (1, 1)):
                            nc.tensor.matmul(
                                pvt[(hh, c)][0 : DH + 1, :],
                                v[st2][:, 2 * j + hh, :],
                                pts[(hh, st2)][:, c * 512 : (c + 1) * 512],
                                start=(st2 == 0),
                                stop=(st2 == KD - 1),
                            )

                    for st in range(KD):
                        pta = ptpool.tile([P, TB], F16, tag=f"pt{st}", name="pta")
                        ptb = ptpool.tile([P, TB], F16, tag=f"pt{st}", name="ptb")
                        pts[(0, st)] = pta
                        pts[(1, st)] = ptb
                        # c0 pair: row-groups (0,0) / (64,0) co-issue
                        sA = psc.tile([P, 512], F32, tag="sc", name="sA")
                        sB = psc.tile([P, 512], F32, tag="sc", name="sB")
                        nc.tensor.matmul(
                            sA[:], ktj[0:DH, st * P : (st + 1) * P], qtj[0:DH, 0:512],
                            start=True, stop=True,
                        )
                        nc.tensor.matmul(
                            sB[:], ktj[DH : 2 * DH, st * P : (st + 1) * P],
                            qtj[DH : 2 * DH, 0:512], start=True, stop=True,
                        )
                        nc.scalar.activation(pta[:, 0:512], sA[:], EXPF, scale=0.125)
                        nc.scalar.activation(ptb[:, 0:512], sB[:], EXPF, scale=0.125)
                        if st > 0:
                            pv_mms(st - 1)

                        def c1_pair(pta=pta, ptb=ptb, st=st):
                            sC = psc.tile([P, 512], F32, tag="sc", name="sC")
                            sD = psc.tile([P, 512], F32, tag="sc", name="sD")
                            nc.tensor.matmul(
                                sC[:], ktj[0:DH, st * P : (st + 1) * P],
                                qtj[0:DH, 512:TB], start=True, stop=True,
                            )
                            nc.tensor.matmul(
                                sD[:], ktj[DH : 2 * DH, st * P : (st + 1) * P],
                                qtj[DH : 2 * DH, 512:TB], start=True, stop=True,
                            )
                            nc.scalar.activation(pta[:, 512:TB], sC[:], EXPF, scale=0.125)
                            nc.scalar.activation(ptb[:, 512:TB], sD[:], EXPF, scale=0.125)

                        # chains between the pairs give the c1 pair's PSUM
                        # WAR (on this st's sA slot) time to clear -> co-issue.
                        # Last st: c1 pair first so its exps are ready for the
                        # post-loop pv_mms of st 7.
                        nslots = KD - st
                        npop = (len(chl) + nslots - 1) // nslots if chl else 0
                        if st == KD - 1:
                            c1_pair()
                        for chn in chl[:npop]:
                            chn.run()
                        del chl[:npop]
                        if st < KD - 1:
                            c1_pair()
                    pv_mms(KD - 1)
                    if DEBUG_DUMPS and b == 0 and j == 0:
                        nc.sync.dma_start(dbg["d_qt"][:, :], qtj[:])
                        nc.sync.dma_start(dbg["d_kt"][:, :], ktj[:])
                        nc.sync.dma_start(dbg["d_pt"][:, :], pts[(0, 0)][:])
                        nc.sync.dma_start(
                            dbg["d_v0"][:, :], v[0].rearrange("p h d -> p (h d)")
                        )

            # ---- tail: last pending normalize + out-proj of last batch ----
            drain_norm(pending)
            for i, chn in enumerate(out_chains(NB - 1, ctxt)):
                chn.bank = ("br", "pv00", "pv10")[i % 3]
                chn.run()

    nc.compile()
    return nc


def _get_nc():
    global _NC_CACHE
    if _NC_CACHE is None:
        _NC_CACHE = build_nc()
    return _NC_CACHE


def kernel(hidden_states, Wq, bq, Wk, bk, Wv, bv, Wo, bo):
    global LAST_RESULTS
    f16 = np.float16
    hs = np.asarray(hidden_states, np.float32)
    Wq = np.asarray(Wq, np.float32)
    Wk = np.asarray(Wk, np.float32)
    Wv = np.asarray(Wv, np.float32)
    Wo = np.asarray(Wo, np.float32)
    bq = np.asarray(bq, np.float32)
    bk = np.asarray(bk, np.float32)
    bv = np.asarray(bv, np.float32)
    bo = np.asarray(bo, np.float32)

    wqt = np.ascontiguousarray(Wq.T).astype(f16)
    wkt = np.ascontiguousarray(Wk.T).astype(f16)
    wvt = np.ascontiguousarray(Wv.T).astype(f16)
    wot = np.ascontiguousarray(Wo.T).astype(f16)
    bo2 = (bo + Wo @ bv).astype(np.float32)

    bpc = hs.shape[0] // NCORES  # batches per core
    in_maps = []
    for c in range(NCORES):
        xc = hs[c * bpc : (c + 1) * bpc].reshape(bpc * TB, D)
        in_maps.append(
            {
                "xt": np.ascontiguousarray(xc.T).astype(f16),
                "wqt": wqt,
                "wkt": wkt,
                "wvt": wvt,
                "wot": wot,
                "bq": bq,
                "bk": bk,
                "bo2": bo2,
            }
        )

    nc = _get_nc()
    res = run_bass_kernel_spmd(
        nc,
        in_maps,
        core_ids=list(range(NCORES)),
        trace=TRACE,
        **TRACE_KWARGS,
    )
    LAST_RESULTS = res

    out = np.empty((hs.shape[0], TB, D), np.float32)
    for c in range(NCORES):
        ot = res.results[c]["outt"]  # [D, T]
        for b in range(bpc):
            out[c * bpc + b] = ot[:, b * TB : (b + 1) * TB].T
    return out


# revision 23
# speedup vs baseline: 1.0562x; 1.0562x over previous
"""Multi-head attention forward (B=16, S=1024, d=1024, H=16, Dh=64) on 8
Trainium2 NeuronCores, data-parallel over batch (2 batches per core).

v3: fp16 matmuls (fp32 accumulate), fully chain-structured PE stream.
  - scores per head-pair j: the two K=64 matmuls (head h0 on PE rows 0-63,
    h1 on rows 64-127) write the two banks of one 2-bank-wide PSUM tile and
    co-issue via PE row tiling; ONE 1024-wide exp per pair-chunk halves the
    scalar-engine instruction count.
  - every projection (Q/K/V/O) and the PV accumulation are 8-matmul
    single-bank accumulation chains; PV chains for pair j run during pair
    j+1 (exp outputs are retained one generation), so nothing couples the
    PE issue rate to exp latency.
  - PSUM: 4 banks scores (2x wide tiles, constant slots), 2 PV chain banks,
    1 projection chain bank, 1 spare.

Device layout per core:
  XT [d, 2048] = hidden[2c:2c+2].reshape(2048,d).T  (fp16)
  WqT/WkT/WvT/WoT = W.T [in, out] fp16; bq, bk [1024] f32;
  bo2 = bo + Wo @ bv (bv folded: softmax rows sum to 1).
  QT/KT per pair j: [128, 1024];  V head-split with ones column;
  PT per (j, st): [128, 2048] fp16 = [h0c0 | h1c0 | h0c1 | h1c1];
  ctxT_aug[dv+1, t] accumulated in PSUM, row dv = softmax denominator.
  outT[o, t] = WoT.T @ ctxT (+bo2) -> host transposes back.
"""

import numpy as np
import ml_dtypes

import concourse.bass as bass
import concourse.mybir as mybir
import concourse.tile as tile
from concourse import bacc
from concourse.bass_utils import run_bass_kernel_spmd

P = 128
D = 1024
T = 2048  # tokens per core
TB = 1024  # tokens per batch (= S)
H = 16
DH = 64
KD = D // P  # 8 partition-tiles of the d/dv/s dims
NB = T // TB  # batches per core
NCORES = 8

F16 = mybir.dt.float16
F32 = mybir.dt.float32
EXPF = mybir.ActivationFunctionType.Exp
IDF = mybir.ActivationFunctionType.Identity
EXPF_SET = None
MULT = mybir.AluOpType.mult

# test.py hooks
TRACE = False
TRACE_KWARGS = {}
LAST_RESULTS = None
DEBUG_DUMPS = False

_NC_CACHE = None


def build_nc():
    nc = bacc.Bacc("TRN2", target_bir_lowering=False, debug=False, num_devices=NCORES)

    xt_d = nc.dram_tensor("xt", [D, T], F16, kind="ExternalInput")
    wqt_d = nc.dram_tensor("wqt", [D, D], F16, kind="ExternalInput")
    wkt_d = nc.dram_tensor("wkt", [D, D], F16, kind="ExternalInput")
    wvt_d = nc.dram_tensor("wvt", [D, D], F16, kind="ExternalInput")
    wot_d = nc.dram_tensor("wot", [D, D], F16, kind="ExternalInput")
    bq_d = nc.dram_tensor("bq", [D], F32, kind="ExternalInput")
    bk_d = nc.dram_tensor("bk", [D], F32, kind="ExternalInput")
    bo2_d = nc.dram_tensor("bo2", [D], F32, kind="ExternalInput")
    outt_d = nc.dram_tensor("outt", [D, T], F32, kind="ExternalOutput")

    with tile.TileContext(nc) as tc:
        from contextlib import ExitStack

        with ExitStack() as ctx:
            wpool = ctx.enter_context(tc.tile_pool(name="w", bufs=1))
            xpool = ctx.enter_context(tc.tile_pool(name="x", bufs=1))
            qkpool = ctx.enter_context(tc.tile_pool(name="qk", bufs=2))
            vpool = ctx.enter_context(tc.tile_pool(name="v", bufs=2))
            ptpool = ctx.enter_context(tc.tile_pool(name="pt", bufs=1))
            cpool = ctx.enter_context(tc.tile_pool(name="ctx", bufs=1))
            spool = ctx.enter_context(tc.tile_pool(name="small", bufs=1))
            npool = ctx.enter_context(tc.tile_pool(name="norm", bufs=2))
            upool = ctx.enter_context(tc.tile_pool(name="unorm", bufs=1))
            opool = ctx.enter_context(tc.tile_pool(name="out", bufs=1))
            # PSUM: "sc" 2x[128,1024] (4 banks) + pv0/pv1 + br + spare "x1"
            psc = ctx.enter_context(tc.tile_pool(name="psc", bufs=2, space="PSUM"))
            ppv = ctx.enter_context(tc.tile_pool(name="ppv", bufs=1, space="PSUM"))
            pbr = ctx.enter_context(tc.tile_pool(name="pbr", bufs=1, space="PSUM"))

            # ---- global loads (biases first: needed by earliest drains) ----
            bq_sb = spool.tile([P, KD], F32, tag="bq", name="bq_sb")
            bk_sb = spool.tile([P, KD], F32, tag="bk", name="bk_sb")
            bo_sb = spool.tile([P, KD], F32, tag="bo", name="bo_sb")
            for sb, dd in ((bq_sb, bq_d), (bk_sb, bk_d), (bo_sb, bo2_d)):
                nc.sync.dma_start(sb[:], dd.rearrange("(o p) -> p o", p=P))

            xt = [xpool.tile([P, T], F16, tag=f"xt{k}", name=f"xt{k}") for k in range(KD)]
            wq, wk, wv, wo = (
                [wpool.tile([P, D], F16, tag=f"w{nm}{k}", name=f"w{nm}{k}") for k in range(KD)]
                for nm in "qkvo"
            )
            for k in range(KD):
                nc.sync.dma_start(wv[k][:], wvt_d[k * P : (k + 1) * P, :])
                nc.sync.dma_start(xt[k][:], xt_d[k * P : (k + 1) * P, :])
            for wt, wd in ((wq, wqt_d), (wk, wkt_d), (wo, wot_d)):
                for k in range(KD):
                    nc.sync.dma_start(wt[k][:], wd[k * P : (k + 1) * P, :])

            # ---- 8-matmul single-bank accumulation chains ----
            class Chain:
                def __init__(self, mk_mm, drain, bank="br"):
                    self.mk_mm = mk_mm
                    self.drain = drain
                    self.bank = bank

                def run(self):
                    pool = pbr if self.bank == "br" else ppv
                    if self.bank == "sc":
                        ps = psc.tile([P, TB], F32, tag="sc", name="chps")[:, 0:512]
                    else:
                        ps = pool.tile([P, 512], F32, tag=self.bank, name="chps")
                    for k in range(KD):
                        self.mk_mm(k, ps)
                    self.drain(ps)

            def qk_chains(b2, j2):
                qtj = qkpool.tile([P, TB], F16, tag="qtj", name="qtj")
                ktj = qkpool.tile([P, TB], F16, tag="ktj", name="ktj")
                chains = []
                for wt, bias, dest in ((wq, bq_sb, qtj), (wk, bk_sb, ktj)):
                    for c in range(2):
                        def mk(k, ps, wt=wt, c=c):
                            nc.tensor.matmul(
                                ps[:],
                                wt[k][:, j2 * P : (j2 + 1) * P],
                                xt[k][:, b2 * TB + c * 512 : b2 * TB + (c + 1) * 512],
                                start=(k == 0),
                                stop=(k == KD - 1),
                            )

                        def dr(ps, dest=dest, c=c, bias=bias):
                            nc.vector.tensor_scalar_add(
                                dest[:, c * 512 : (c + 1) * 512], ps[:], bias[:, j2 : j2 + 1]
                            )

                        chains.append(Chain(mk, dr))
                return (qtj, ktj), chains

            def v_chains(b2):
                vt = [
                    vpool.tile([P, H, DH + 1], F16, tag=f"v{mt}", name=f"v{mt}")
                    for mt in range(KD)
                ]
                for mt in range(KD):
                    nc.vector.memset(vt[mt][:, :, DH : DH + 1], 1.0)
                chains = []
                for mt in range(KD):
                    for c in range(2):
                        def mk(k, ps, mt=mt, c=c):
                            nc.tensor.matmul(
                                ps[:],
                                xt[k][:, (b2 * KD + mt) * P : (b2 * KD + mt + 1) * P],
                                wv[k][:, c * 512 : (c + 1) * 512],
                                start=(k == 0),
                                stop=(k == KD - 1),
                            )

                        def dr(ps, mt=mt, c=c):
                            nc.vector.tensor_copy(
                                vt[mt][:, c * 8 : (c + 1) * 8, 0:DH],
                                ps.rearrange("p (h d) -> p h d", d=DH),
                            )

                        chains.append(Chain(mk, dr))
                return vt, chains

            def out_chains(b2, tail=False):
                chains = []
                for mo in range(KD):
                    for c in range(2):
                        def mk(k, ps, mo=mo, c=c):
                            nc.tensor.matmul(
                                ps[:],
                                wo[k][:, mo * P : (mo + 1) * P],
                                ctxt[k][:, c * 512 : (c + 1) * 512],
                                start=(k == 0),
                                stop=(k == KD - 1),
                            )

                        def dr(ps, mo=mo, c=c, tail=tail):
                            osb = opool.tile([P, 512], F32, tag="osb", name="osb")
                            if tail:
                                # scalar engine is idle in the tail
                                nc.scalar.activation(
                                    osb[:], ps[:], IDF, bias=bo_sb[:, mo : mo + 1]
                                )
                            else:
                                nc.vector.tensor_scalar_add(
                                    osb[:], ps[:], bo_sb[:, mo : mo + 1]
                                )
                            nc.sync.dma_start(
                                outt_d[
                                    mo * P : (mo + 1) * P,
                                    b2 * TB + c * 512 : b2 * TB + (c + 1) * 512,
                                ],
                                osb[:],
                            )

                        chains.append(Chain(mk, dr))
                return chains

            def pv_chains(j2, pts, v):
                """PV for pair j2, consuming the retained pt tiles; each
                (head, chunk) is one 8-matmul chain ending in normalize."""
                chains = []
                for hh in range(2):
                    for c in range(2):
                        def mk(k, ps, hh=hh, c=c, pts=pts, v=v, j2=j2):
                            nc.tensor.matmul(
                                ps[0 : DH + 1, :],
                                v[k][:, 2 * j2 + hh, :],
                                pts[k][:, c * TB + hh * 512 : c * TB + (hh + 1) * 512],
                                start=(k == 0),
                                stop=(k == KD - 1),
                            )

                        def dr(ps, hh=hh, c=c, j2=j2):
                            # single u tag: all producers/consumers are DVE,
                            # in-order execution makes reuse safe
                            u = upool.tile([DH + 1, 512], F32, tag="u", name="u")
                            nc.vector.tensor_copy(u[:], ps[0 : DH + 1, :])
                            rs = npool.tile([1, 512], F32, tag="rs", name="rs", bufs=1)
                            nc.vector.tensor_copy(rs[:], u[DH : DH + 1, :])
                            rr = npool.tile([1, 512], F32, tag="rr", name="rr", bufs=1)
                            nc.vector.reciprocal_approx_fast(rr[:], rs[:])
                            rb = npool.tile([DH, 512], F32, tag="rb", name="rb", bufs=1)
                            nc.gpsimd.partition_broadcast(rb[:], rr[:])
                            if hh == 0:
                                nc.vector.tensor_tensor(
                                    ctxt[j2][0:DH, c * 512 : (c + 1) * 512],
                                    u[0:DH, :], rb[:], MULT,
                                )
                            else:
                                ch = npool.tile([DH, 512], F16, tag="ch", name="ch")
                                nc.vector.tensor_tensor(ch[:], u[0:DH, :], rb[:], MULT)
                                nc.sync.dma_start(
                                    ctxt[j2][DH : 2 * DH, c * 512 : (c + 1) * 512], ch[:]
                                )

                        chains.append(Chain(mk, dr))
                return chains

            qk_store = {}
            v_store = {}
            pv_prev = []  # pv chains of the previous pair, run this pair

            # ctxt tiles shared across batches: normalize(b+1) writes get a
            # WAR dependency on out-proj(b) reads of the same tile.
            ctxt = [
                cpool.tile([P, TB], F16, tag=f"ctxt{m}", name=f"ctxt{m}")
                for m in range(KD)
            ]

            # ---- prologue: v(b0) mt0 + qk(b0, j0) ----
            vt0, vch0 = v_chains(0)
            v_store[0] = vt0
            qk_store[(0, 0)], qkch0 = qk_chains(0, 0)
            rest_v = vch0[2:]
            for i, chn in enumerate(vch0[0:2] + qkch0):
                chn.bank = ("br", "pv0", "pv1")[i % 3]
                chn.run()

            out_rest = []  # out chains of prev batch, spread over j1..j2
            for b in range(NB):
                for j in range(KD):
                    qtj, ktj = qk_store.pop((b, j))
                    v = v_store[b]

                    # pv chains of the previous pair run as a burst BEFORE
                    # this j's st loop: all their pt reads are emitted before
                    # the bufs=1 pt tags are re-allocated below, and ctxt of
                    # the previous pair completes early.
                    for i, chn in enumerate(pv_prev):
                        chn.bank = ("pv0", "pv1")[i % 2]
                        chn.run()
                    pv_prev = []

                    # projection chain list for this j
                    chl = []
                    if b == 0 and j == 0:
                        chl += rest_v
                    if j < KD - 1:
                        qk_store[(b, j + 1)], ch = qk_chains(b, j + 1)
                        chl += ch
                    elif b + 1 < NB:
                        qk_store[(b + 1, 0)], ch = qk_chains(b + 1, 0)
                        chl += ch
                    # out-proj of the previous batch: must be fully emitted
                    # within j0, before normalize of THIS batch's pair 0
                    # overwrites the shared ctxt tiles at the j1 top-burst.
                    if b > 0 and j == 0:
                        chl += out_rest
                        out_rest = []
                    if b + 1 < NB and j == 4:
                        vtn, vchn = v_chains(b + 1)
                        v_store[b + 1] = vtn
                        v_store["rest"] = vchn
                    if b + 1 < NB and 4 <= j:
                        vr = v_store.get("rest", [])
                        ntk = -(-len(vr) // (KD - j)) if vr else 0
                        chl += vr[:ntk]
                        del vr[:ntk]

                    pts = {}
                    for st in range(KD):
                        # st0 double-buffered: its exp is the first consumer
                        # of the new generation while the pv burst of the old
                        # pair is still reading the old one
                        pt = ptpool.tile(
                            [P, 2 * TB], F16, tag=f"pt{st}", name="pt",
                            bufs=2 if st == 0 else 1,
                        )
                        pts[st] = pt
                        # c0 pair: W0 wide tile, h0 -> bank A rows 0-63,
                        # h1 -> bank B rows 64-127 (co-issue via row tiling)
                        w0 = psc.tile([P, TB], F32, tag="sc", name="w0")
                        nc.tensor.matmul(
                            w0[:, 0:512], ktj[0:DH, st * P : (st + 1) * P],
                            qtj[0:DH, 0:512], start=True, stop=True,
                        )
                        nc.tensor.matmul(
                            w0[:, 512:TB], ktj[DH : 2 * DH, st * P : (st + 1) * P],
                            qtj[DH : 2 * DH, 0:512], start=True, stop=True,
                        )
                        nc.scalar.activation(pt[:, 0:TB], w0[:], EXPF, scale=0.125)

                        # one projection/pv chain between the pairs
                        nslots = KD - st
                        npop = (len(chl) + nslots - 1) // nslots if chl else 0
                        for chn in chl[:npop]:
                            chn.run()
                        del chl[:npop]

                        # c1 pair
                        w1 = psc.tile([P, TB], F32, tag="sc", name="w1")
                        nc.tensor.matmul(
                            w1[:, 0:512], ktj[0:DH, st * P : (st + 1) * P],
                            qtj[0:DH, 512:TB], start=True, stop=True,
                        )
                        nc.tensor.matmul(
                            w1[:, 512:TB], ktj[DH : 2 * DH, st * P : (st + 1) * P],
                            qtj[DH : 2 * DH, 512:TB], start=True, stop=True,
                        )
                        nc.scalar.activation(pt[:, TB : 2 * TB], w1[:], EXPF, scale=0.125)

                    pv_prev = pv_chains(j, pts, v)

                    if DEBUG_DUMPS and b == 0 and j == 0:
                        nc.sync.dma_start(dbg["d_qt"][:, :], qtj[:])
                        nc.sync.dma_start(dbg["d_kt"][:, :], ktj[:])
                        nc.sync.dma_start(dbg["d_pt"][:, 0:512], pts[0][:, 0:512])
                        nc.sync.dma_start(dbg["d_pt"][:, 512:TB], pts[0][:, TB : TB + 512])
                        nc.sync.dma_start(
                            dbg["d_v0"][:, :], v[0].rearrange("p h d -> p (h d)")
                        )

                # batch boundary: queue out chains of this batch
                if b + 1 < NB:
                    out_rest = out_chains(b)

            # ---- tail: pv(b1, j7) + out-proj of last batch ----
            for i, chn in enumerate(pv_prev):
                chn.bank = ("pv0", "pv1")[i % 2]
                chn.run()
            for i, chn in enumerate(out_chains(NB - 1, tail=True)):
                chn.bank = ("br", "sc", "pv0", "pv1")[i % 4]
                chn.run()

    nc.compile()
    return nc


def _get_nc():
    global _NC_CACHE
    if _NC_CACHE is None:
        _NC_CACHE = build_nc()
    return _NC_CACHE


def kernel(hidden_states, Wq, bq, Wk, bk, Wv, bv, Wo, bo):
    global LAST_RESULTS
    f16 = np.float16
    hs = np.asarray(hidden_states, np.float32)
    Wq = np.asarray(Wq, np.float32)
    Wk = np.asarray(Wk, np.float32)
    Wv = np.asarray(Wv, np.float32)
    Wo = np.asarray(Wo, np.float32)
    bq = np.asarray(bq, np.float32)
    bk = np.asarray(bk, np.float32)
    bv = np.asarray(bv, np.float32)
    bo = np.asarray(bo, np.float32)

    wqt = np.ascontiguousarray(Wq.T).astype(f16)
    wkt = np.ascontiguousarray(Wk.T).astype(f16)
    wvt = np.ascontiguousarray(Wv.T).astype(f16)
    wot = np.ascontiguousarray(Wo.T).astype(f16)
    bo2 = (bo + Wo @ bv).astype(np.float32)

    bpc = hs.shape[0] // NCORES  # batches per core
    in_maps = []
    for c in range(NCORES):
        xc = hs[c * bpc : (c + 1) * bpc].reshape(bpc * TB, D)
        in_maps.append(
            {
                "xt": np.ascontiguousarray(xc.T).astype(f16),
                "wqt": wqt,
                "wkt": wkt,
                "wvt": wvt,
                "wot": wot,
                "bq": bq,
                "bk": bk,
                "bo2": bo2,
            }
        )

    nc = _get_nc()
    res = run_bass_kernel_spmd(
        nc,
        in_maps,
        core_ids=list(range(NCORES)),
        trace=TRACE,
        **TRACE_KWARGS,
    )
    LAST_RESULTS = res

    out = np.empty((hs.shape[0], TB, D), np.float32)
    for c in range(NCORES):
        ot = res.results[c]["outt"]  # [D, T]
        for b in range(bpc):
            out[c * bpc + b] = ot[:, b * TB : (b + 1) * TB].T
    return out


# revision 26
# speedup vs baseline: 1.3379x; 1.2667x over previous
"""Multi-head attention forward (B=16, S=1024, d=1024, H=16, Dh=64) on 8
Trainium2 NeuronCores, data-parallel over batch (2 batches per core).

v3: fp16 matmuls (fp32 accumulate), fully chain-structured PE stream.
  - scores per head-pair j: the two K=64 matmuls (head h0 on PE rows 0-63,
    h1 on rows 64-127) write the two banks of one 2-bank-wide PSUM tile and
    co-issue via PE row tiling; ONE 1024-wide exp per pair-chunk halves the
    scalar-engine instruction count.
  - every projection (Q/K/V/O) and the PV accumulation are 8-matmul
    single-bank accumulation chains; PV chains for pair j run during pair
    j+1 (exp outputs are retained one generation), so nothing couples the
    PE issue rate to exp latency.
  - PSUM: 4 banks scores (2x wide tiles, constant slots), 2 PV chain banks,
    1 projection chain bank, 1 spare.

Device layout per core:
  XT [d, 2048] = hidden[2c:2c+2].reshape(2048,d).T  (fp16)
  WqT/WkT/WvT/WoT = W.T [in, out] fp16; bq, bk [1024] f32;
  bo2 = bo + Wo @ bv (bv folded: softmax rows sum to 1).
  QT/KT per pair j: [128, 1024];  V head-split with ones column;
  PT per (j, st): [128, 2048] fp16 = [h0c0 | h1c0 | h0c1 | h1c1];
  ctxT_aug[dv+1, t] accumulated in PSUM, row dv = softmax denominator.
  outT[o, t] = WoT.T @ ctxT (+bo2) -> host transposes back.
"""

import numpy as np
import ml_dtypes

import concourse.bass as bass
import concourse.mybir as mybir
import concourse.tile as tile
from concourse import bacc
from concourse.bass_utils import run_bass_kernel_spmd

P = 128
D = 1024
T = 2048  # tokens per core
TB = 1024  # tokens per batch (= S)
H = 16
DH = 64
KD = D // P  # 8 partition-tiles of the d/dv/s dims
NB = T // TB  # batches per core
NCORES = 8

F16 = mybir.dt.float16
F32 = mybir.dt.float32
EXPF = mybir.ActivationFunctionType.Exp
IDF = mybir.ActivationFunctionType.Identity
EXPF_SET = None
MULT = mybir.AluOpType.mult

# test.py hooks
TRACE = False
TRACE_KWARGS = {}
LAST_RESULTS = None
DEBUG_DUMPS = False

_NC_CACHE = None


def build_nc():
    nc = bacc.Bacc("TRN2", target_bir_lowering=False, debug=False, num_devices=NCORES)

    xt_d = nc.dram_tensor("xt", [D, T], F16, kind="ExternalInput")
    wqt_d = nc.dram_tensor("wqt", [D, D], F16, kind="ExternalInput")
    wkt_d = nc.dram_tensor("wkt", [D, D], F16, kind="ExternalInput")
    wvt_d = nc.dram_tensor("wvt", [D, D], F16, kind="ExternalInput")
    wot_d = nc.dram_tensor("wot", [D, D], F16, kind="ExternalInput")
    bq_d = nc.dram_tensor("bq", [D], F32, kind="ExternalInput")
    bk_d = nc.dram_tensor("bk", [D], F32, kind="ExternalInput")
    bo2_d = nc.dram_tensor("bo2", [D], F32, kind="ExternalInput")
    outt_d = nc.dram_tensor("outt", [D, T], F32, kind="ExternalOutput")

    with tile.TileContext(nc) as tc:
        from contextlib import ExitStack

        with ExitStack() as ctx:
            wpool = ctx.enter_context(tc.tile_pool(name="w", bufs=1))
            xpool = ctx.enter_context(tc.tile_pool(name="x", bufs=1))
            qkpool = ctx.enter_context(tc.tile_pool(name="qk", bufs=2))
            vpool = ctx.enter_context(tc.tile_pool(name="v", bufs=2))
            ptpool = ctx.enter_context(tc.tile_pool(name="pt", bufs=1))
            cpool = ctx.enter_context(tc.tile_pool(name="ctx", bufs=1))
            spool = ctx.enter_context(tc.tile_pool(name="small", bufs=1))
            npool = ctx.enter_context(tc.tile_pool(name="norm", bufs=2))
            upool = ctx.enter_context(tc.tile_pool(name="unorm", bufs=1))
            opool = ctx.enter_context(tc.tile_pool(name="out", bufs=1))
            # PSUM: "sc" 2x[128,1024] (4 banks) + pv0/pv1 + br + spare "x1"
            psc = ctx.enter_context(tc.tile_pool(name="psc", bufs=2, space="PSUM"))
            ppv = ctx.enter_context(tc.tile_pool(name="ppv", bufs=1, space="PSUM"))
            pbr = ctx.enter_context(tc.tile_pool(name="pbr", bufs=1, space="PSUM"))

            # ---- global loads (biases first: needed by earliest drains) ----
            bq_sb = spool.tile([P, KD], F32, tag="bq", name="bq_sb")
            bk_sb = spool.tile([P, KD], F32, tag="bk", name="bk_sb")
            bo_sb = spool.tile([P, KD], F32, tag="bo", name="bo_sb")
            for sb, dd in ((bq_sb, bq_d), (bk_sb, bk_d), (bo_sb, bo2_d)):
                nc.sync.dma_start(sb[:], dd.rearrange("(o p) -> p o", p=P))

            xt = [xpool.tile([P, T], F16, tag=f"xt{k}", name=f"xt{k}") for k in range(KD)]
            wq, wk, wv, wo = (
                [wpool.tile([P, D], F16, tag=f"w{nm}{k}", name=f"w{nm}{k}") for k in range(KD)]
                for nm in "qkvo"
            )
            for k in range(KD):
                nc.sync.dma_start(wv[k][:], wvt_d[k * P : (k + 1) * P, :])
                nc.sync.dma_start(xt[k][:], xt_d[k * P : (k + 1) * P, :])
            for wt, wd in ((wq, wqt_d), (wk, wkt_d), (wo, wot_d)):
                for k in range(KD):
                    nc.sync.dma_start(wt[k][:], wd[k * P : (k + 1) * P, :])

            # ---- 8-matmul single-bank accumulation chains ----
            # projection chains ping-pong between the "br" and "x1" banks so
            # a chain's first matmul never waits on the previous chain's
            # DVE drain (a serialized drain also drops the PE p-state)
            br_cycle = [0]

            class Chain:
                def __init__(self, mk_mm, drain, bank=None):
                    self.mk_mm = mk_mm
                    self.drain = drain
                    self.bank = bank

                def run(self):
                    bank = self.bank
                    if bank is None:
                        bank = ("br", "x1")[br_cycle[0] % 2]
                        br_cycle[0] += 1
                    if bank == "sc":
                        ps = psc.tile([P, TB], F32, tag="sc", name="chps")[:, 0:512]
                    elif bank in ("br", "x1"):
                        ps = pbr.tile([P, 512], F32, tag=bank, name="chps")
                    else:
                        ps = ppv.tile([P, 512], F32, tag=bank, name="chps")
                    for k in range(KD):
                        self.mk_mm(k, ps)
                    self.drain(ps)

            def qk_chains(b2, j2):
                qtj = qkpool.tile([P, TB], F16, tag="qtj", name="qtj")
                ktj = qkpool.tile([P, TB], F16, tag="ktj", name="ktj")
                chains = []
                for wt, bias, dest in ((wq, bq_sb, qtj), (wk, bk_sb, ktj)):
                    for c in range(2):
                        def mk(k, ps, wt=wt, c=c):
                            nc.tensor.matmul(
                                ps[:],
                                wt[k][:, j2 * P : (j2 + 1) * P],
                                xt[k][:, b2 * TB + c * 512 : b2 * TB + (c + 1) * 512],
                                start=(k == 0),
                                stop=(k == KD - 1),
                            )

                        def dr(ps, dest=dest, c=c, bias=bias):
                            nc.vector.tensor_scalar_add(
                                dest[:, c * 512 : (c + 1) * 512], ps[:], bias[:, j2 : j2 + 1]
                            )

                        chains.append(Chain(mk, dr))
                return (qtj, ktj), chains

            def v_chains(b2):
                vt = [
                    vpool.tile([P, H, DH + 1], F16, tag=f"v{mt}", name=f"v{mt}")
                    for mt in range(KD)
                ]
                for mt in range(KD):
                    nc.vector.memset(vt[mt][:, :, DH : DH + 1], 1.0)
                chains = []
                for mt in range(KD):
                    for c in range(2):
                        def mk(k, ps, mt=mt, c=c):
                            nc.tensor.matmul(
                                ps[:],
                                xt[k][:, (b2 * KD + mt) * P : (b2 * KD + mt + 1) * P],
                                wv[k][:, c * 512 : (c + 1) * 512],
                                start=(k == 0),
                                stop=(k == KD - 1),
                            )

                        def dr(ps, mt=mt, c=c):
                            nc.vector.tensor_copy(
                                vt[mt][:, c * 8 : (c + 1) * 8, 0:DH],
                                ps.rearrange("p (h d) -> p h d", d=DH),
                            )

                        chains.append(Chain(mk, dr))
                return vt, chains

            def out_chains(b2, tail=False):
                chains = []
                for mo in range(KD):
                    for c in range(2):
                        def mk(k, ps, mo=mo, c=c):
                            nc.tensor.matmul(
                                ps[:],
                                wo[k][:, mo * P : (mo + 1) * P],
                                ctxt[k][:, c * 512 : (c + 1) * 512],
                                start=(k == 0),
                                stop=(k == KD - 1),
                            )

                        def dr(ps, mo=mo, c=c, tail=tail):
                            osb = opool.tile([P, 512], F32, tag="osb", name="osb")
                            if tail:
                                # scalar engine is idle in the tail
                                nc.scalar.activation(
                                    osb[:], ps[:], IDF, bias=bo_sb[:, mo : mo + 1]
                                )
                            else:
                                nc.vector.tensor_scalar_add(
                                    osb[:], ps[:], bo_sb[:, mo : mo + 1]
                                )
                            nc.sync.dma_start(
                                outt_d[
                                    mo * P : (mo + 1) * P,
                                    b2 * TB + c * 512 : b2 * TB + (c + 1) * 512,
                                ],
                                osb[:],
                            )

                        chains.append(Chain(mk, dr))
                return chains

            def pv_chains(j2, pts, v):
                """PV for pair j2, consuming the retained pt tiles; each
                (head, chunk) is one 8-matmul chain ending in normalize."""
                chains = []
                for hh in range(2):
                    for c in range(2):
                        def mk(k, ps, hh=hh, c=c, pts=pts, v=v, j2=j2):
                            nc.tensor.matmul(
                                ps[0 : DH + 1, :],
                                v[k][:, 2 * j2 + hh, :],
                                pts[k][:, c * TB + hh * 512 : c * TB + (hh + 1) * 512],
                                start=(k == 0),
                                stop=(k == KD - 1),
                            )

                        def dr(ps, hh=hh, c=c, j2=j2):
                            # single u tag: all producers/consumers are DVE,
                            # in-order execution makes reuse safe
                            u = upool.tile([DH + 1, 512], F32, tag="u", name="u")
                            nc.vector.tensor_copy(u[:], ps[0 : DH + 1, :])
                            rs = npool.tile([1, 512], F32, tag="rs", name="rs", bufs=1)
                            nc.vector.tensor_copy(rs[:], u[DH : DH + 1, :])
                            rr = npool.tile([1, 512], F32, tag="rr", name="rr", bufs=1)
                            nc.vector.reciprocal_approx_fast(rr[:], rs[:])
                            rb = npool.tile([DH, 512], F32, tag="rb", name="rb", bufs=1)
                            nc.gpsimd.partition_broadcast(rb[:], rr[:])
                            if hh == 0:
                                nc.vector.tensor_tensor(
                                    ctxt[j2][0:DH, c * 512 : (c + 1) * 512],
                                    u[0:DH, :], rb[:], MULT,
                                )
                            else:
                                ch = npool.tile([DH, 512], F16, tag="ch", name="ch")
                                nc.vector.tensor_tensor(ch[:], u[0:DH, :], rb[:], MULT)
                                nc.sync.dma_start(
                                    ctxt[j2][DH : 2 * DH, c * 512 : (c + 1) * 512], ch[:]
                                )

                        chains.append(Chain(mk, dr))
                return chains

            qk_store = {}
            v_store = {}
            pv_prev = []  # pv chains of the previous pair, run this pair

            # ctxt tiles shared across batches: normalize(b+1) writes get a
            # WAR dependency on out-proj(b) reads of the same tile.
            ctxt = [
                cpool.tile([P, TB], F16, tag=f"ctxt{m}", name=f"ctxt{m}")
                for m in range(KD)
            ]

            # ---- prologue: v(b0) mt0 + qk(b0, j0) ----
            vt0, vch0 = v_chains(0)
            v_store[0] = vt0
            qk_store[(0, 0)], qkch0 = qk_chains(0, 0)
            rest_v = vch0[2:]
            for i, chn in enumerate(vch0[0:2] + qkch0):
                chn.bank = ("br", "x1", "pv0", "pv1")[i % 4]
                chn.run()

            out_rest = []  # out chains of prev batch, spread over j1..j2
            for b in range(NB):
                for j in range(KD):
                    qtj, ktj = qk_store.pop((b, j))
                    v = v_store[b]

                    # pv chains of the previous pair run as a burst BEFORE
                    # this j's st loop: all their pt reads are emitted before
                    # the bufs=1 pt tags are re-allocated below, and ctxt of
                    # the previous pair completes early.
                    for i, chn in enumerate(pv_prev):
                        chn.bank = ("pv0", "pv1")[i % 2]
                        chn.run()
                    pv_prev = []

                    # projection chain list for this j
                    chl = []
                    if b == 0 and j == 0:
                        chl += rest_v
                    if j < KD - 1:
                        qk_store[(b, j + 1)], ch = qk_chains(b, j + 1)
                        chl += ch
                    elif b + 1 < NB:
                        qk_store[(b + 1, 0)], ch = qk_chains(b + 1, 0)
                        chl += ch
                    # out-proj of the previous batch: must be fully emitted
                    # within j0, before normalize of THIS batch's pair 0
                    # overwrites the shared ctxt tiles at the j1 top-burst.
                    if b > 0 and j == 0:
                        chl += out_rest
                        out_rest = []
                    if b + 1 < NB and j == 4:
                        vtn, vchn = v_chains(b + 1)
                        v_store[b + 1] = vtn
                        v_store["rest"] = vchn
                    if b + 1 < NB and 4 <= j:
                        vr = v_store.get("rest", [])
                        ntk = -(-len(vr) // (KD - j)) if vr else 0
                        chl += vr[:ntk]
                        del vr[:ntk]

                    pts = {}
                    for st in range(KD):
                        # st0 double-buffered: its exp is the first consumer
                        # of the new generation while the pv burst of the old
                        # pair is still reading the old one
                        pt = ptpool.tile(
                            [P, 2 * TB], F16, tag=f"pt{st}", name="pt",
                            bufs=2 if st == 0 else 1,
                        )
                        pts[st] = pt
                        # c0 pair: W0 wide tile, h0 -> bank A rows 0-63,
                        # h1 -> bank B rows 64-127 (co-issue via row tiling)
                        w0 = psc.tile([P, TB], F32, tag="sc", name="w0")
                        nc.tensor.matmul(
                            w0[:, 0:512], ktj[0:DH, st * P : (st + 1) * P],
                            qtj[0:DH, 0:512], start=True, stop=True,
                        )
                        nc.tensor.matmul(
                            w0[:, 512:TB], ktj[DH : 2 * DH, st * P : (st + 1) * P],
                            qtj[DH : 2 * DH, 0:512], start=True, stop=True,
                        )
                        nc.scalar.activation(pt[:, 0:TB], w0[:], EXPF, scale=0.125)

                        # one projection/pv chain between the pairs
                        nslots = KD - st
                        npop = (len(chl) + nslots - 1) // nslots if chl else 0
                        for chn in chl[:npop]:
                            chn.run()
                        del chl[:npop]

                        # c1 pair
                        w1 = psc.tile([P, TB], F32, tag="sc", name="w1")
                        nc.tensor.matmul(
                            w1[:, 0:512], ktj[0:DH, st * P : (st + 1) * P],
                            qtj[0:DH, 512:TB], start=True, stop=True,
                        )
                        nc.tensor.matmul(
                            w1[:, 512:TB], ktj[DH : 2 * DH, st * P : (st + 1) * P],
                            qtj[DH : 2 * DH, 512:TB], start=True, stop=True,
                        )
                        nc.scalar.activation(pt[:, TB : 2 * TB], w1[:], EXPF, scale=0.125)

                    pv_prev = pv_chains(j, pts, v)

                    if DEBUG_DUMPS and b == 0 and j == 0:
                        nc.sync.dma_start(dbg["d_qt"][:, :], qtj[:])
                        nc.sync.dma_start(dbg["d_kt"][:, :], ktj[:])
                        nc.sync.dma_start(dbg["d_pt"][:, 0:512], pts[0][:, 0:512])
                        nc.sync.dma_start(dbg["d_pt"][:, 512:TB], pts[0][:, TB : TB + 512])
                        nc.sync.dma_start(
                            dbg["d_v0"][:, :], v[0].rearrange("p h d -> p (h d)")
                        )

                # batch boundary: queue out chains of this batch
                if b + 1 < NB:
                    out_rest = out_chains(b)

            # ---- tail: pv(b1, j7) + out-proj of last batch ----
            for i, chn in enumerate(pv_prev):
                chn.bank = ("pv0", "pv1")[i % 2]
                chn.run()
            for i, chn in enumerate(out_chains(NB - 1, tail=True)):
                chn.bank = ("br", "x1", "sc", "pv0", "pv1")[i % 5]
                chn.run()

    nc.compile()
    return nc


def _get_nc():
    global _NC_CACHE
    if _NC_CACHE is None:
        _NC_CACHE = build_nc()
    return _NC_CACHE


def kernel(hidden_states, Wq, bq, Wk, bk, Wv, bv, Wo, bo):
    global LAST_RESULTS
    f16 = np.float16
    hs = np.asarray(hidden_states, np.float32)
    Wq = np.asarray(Wq, np.float32)
    Wk = np.asarray(Wk, np.float32)
    Wv = np.asarray(Wv, np.float32)
    Wo = np.asarray(Wo, np.float32)
    bq = np.asarray(bq, np.float32)
    bk = np.asarray(bk, np.float32)
    bv = np.asarray(bv, np.float32)
    bo = np.asarray(bo, np.float32)

    wqt = np.ascontiguousarray(Wq.T).astype(f16)
    wkt = np.ascontiguousarray(Wk.T).astype(f16)
    wvt = np.ascontiguousarray(Wv.T).astype(f16)
    wot = np.ascontiguousarray(Wo.T).astype(f16)
    bo2 = (bo + Wo @ bv).astype(np.float32)

    bpc = hs.shape[0] // NCORES  # batches per core
    in_maps = []
    for c in range(NCORES):
        xc = hs[c * bpc : (c + 1) * bpc].reshape(bpc * TB, D)
        in_maps.append(
            {
                "xt": np.ascontiguousarray(xc.T).astype(f16),
                "wqt": wqt,
                "wkt": wkt,
                "wvt": wvt,
                "wot": wot,
                "bq": bq,
                "bk": bk,
                "bo2": bo2,
            }
        )

    nc = _get_nc()
    res = run_bass_kernel_spmd(
        nc,
        in_maps,
        core_ids=list(range(NCORES)),
        trace=TRACE,
        **TRACE_KWARGS,
    )
    LAST_RESULTS = res

    out = np.empty((hs.shape[0], TB, D), np.float32)
    for c in range(NCORES):
        ot = res.results[c]["outt"]  # [D, T]
        for b in range(bpc):
            out[c * bpc + b] = ot[:, b * TB : (b + 1) * TB].T
    return out
